# revision 2
# baseline (speedup 1.0000x reference)
"""GAT 2-layer encoder on 8 Trainium2 NeuronCores.

Reference computation: layer 1 = GAT conv over edge_index[:, :500] (weights W1),
layer 2 = GAT conv over edge_index[:, 500:] (weights W2).

Strategy (constant-default decomposition):
  - Layer-1 output x1 differs from b1 only on the K<=500 distinct dsts of the
    first 500 edges ("specials").  In layer 2, every dst whose in-edges all
    come from default srcs sees identical attention scores -> uniform softmax
    -> output row r1 = b1@W2 + b2, independent of adst.  Only dsts with >=1
    special in-edge (~E*K/N ~ 8000 nodes) need real computation; deg-0 dsts
    get b2 (handled as specials with all-zero weights, via the +EPS in the
    reference's segment softmax).
  - Sharding: dst-range partition across 8 cores.  Layer 1 + the K+1-row
    gather table (512B rows [x1 | x1.(W2 a_src2) | x1.(W2 a_dst2) | pad]) are
    replicated on every core (tiny).
  - Per core and per iteration: one dma_gather for the ~1k special-edge rows
    plus one 512B row per special dst (for adst), a segment softmax with an
    analytic "default mass" pseudo-term (weight c_v = #default in-edges), one
    matmul per 128-dst block, then two big partition-striped DMA writes:
    the special block rows and the broadcast of r1 over all remaining rows.
"""

import sys

sys.path.insert(0, "/opt/trn_rl_repo")

from contextlib import ExitStack

import numpy as np

import concourse.bacc as bacc
import concourse.bass as bass
import concourse.mybir as mybir
import concourse.tile as tile
from concourse.bass_utils import run_bass_kernel_spmd
from concourse.masks import make_identity

F32 = mybir.dt.float32
I16 = mybir.dt.int16
I32 = mybir.dt.int32
AF = mybir.ActivationFunctionType
OP = mybir.AluOpType

N = 100000
D = 64
NCORES = 8
NPC = N // NCORES          # dst nodes per core
P = 128
NSPLIT = 500               # first 500 edges -> layer 1
SMAX = 80                  # max edge-slots per superblock (layer-1 SBUF budget)
NEG_SLOPE = 0.2
EPS = 1e-16
BIG = 200.0                # score shift so padded slots underflow exp to 0.0
VTAB = 1024                # gather table rows (specials + default replicas)


def _wrap16(flat):
    """int16 stream [n] (n%16==0) -> dma_gather idx tile [128, n//16]."""
    w = flat.reshape(-1, 16).T
    return np.ascontiguousarray(np.tile(w, (8, 1)).astype(np.int16))


def _grid(deg_sorted_max, npos):
    """Layer-1 block structure from the descending degree profile."""
    nblocks = npos // P
    L = [max(int(deg_sorted_max[b * P]), 1) for b in range(nblocks)]
    sbs = []
    b = 0
    while b < nblocks:
        s = 0
        b0 = b
        while b < nblocks and (b - b0) < 16 and s + L[b] <= max(SMAX, L[b0]):
            s += L[b]
            b += 1
        sbs.append({"b0": b0, "b1": b, "S": s})
    slot0 = 0
    for sb in sbs:
        sb["slot0"] = slot0
        slot0 += sb["S"]
    groups = []
    for si, sb in enumerate(sbs):
        off = 0
        b = sb["b0"]
        while b < sb["b1"]:
            b0 = b
            while b < sb["b1"] and L[b] == L[b0]:
                b += 1
            groups.append({"sb": si, "b0": b0, "B": b - b0, "L": L[b0], "slot_off": off})
            off += (b - b0) * L[b0]
    return L, sbs, groups


def prep(inputs):
    """Host-side index prep (pure index computation, no feature values)."""
    ei = np.asarray(inputs["edge_index"])
    src = ei[0].astype(np.int64)
    dst = ei[1].astype(np.int64)
    s1, d1 = src[:NSPLIT], dst[:NSPLIT]
    s2, d2 = src[NSPLIT:], dst[NSPLIT:]

    # ---- layer 1 structure ----
    specials, deg1 = np.unique(d1, return_counts=True)
    K = len(specials)
    order1 = np.argsort(-deg1, kind="stable")
    spec_by_pos = specials[order1]          # grid position q -> node, table row q+1
    rowmap = np.zeros(N, np.int16)
    rowmap[spec_by_pos] = np.arange(1, K + 1)
    K1pos = K + 1                            # one guaranteed pad slot (default row)
    nblk1 = (K1pos + P - 1) // P
    npos1 = nblk1 * P

    U = np.unique(np.concatenate([s1, d1]))
    nU = len(U)
    nUt = (nU + P - 1) // P
    uidx = np.zeros((P, nUt), np.int32)
    upad = np.zeros(nUt * P, np.int64)
    upad[:nU] = U
    uidx[:, :] = upad.reshape(nUt, P).T
    uindex = np.zeros(N, np.int64)
    uindex[U] = np.arange(nU)

    # layer-1 edge grid (dst -> grid position via rank over specials)
    rank1 = np.empty(K, np.int64)
    rank1[order1] = np.arange(K)
    d1pos = rank1[np.searchsorted(specials, d1)]
    deg1_sorted = np.zeros(npos1, np.int64)
    deg1_sorted[:K] = deg1[order1]
    L1, sbs1, groups1 = _grid(deg1_sorted, npos1)
    S1 = int(sum(L1))
    pe = np.argsort(d1pos, kind="stable")
    pos_s = d1pos[pe]
    val_s = uindex[s1[pe]].astype(np.int16)
    start_of_pos = np.searchsorted(pos_s, np.arange(npos1))
    k = np.arange(len(pos_s)) - start_of_pos[pos_s]
    slot_base = np.concatenate([[0], np.cumsum(L1)])[:-1]
    flat_j = (slot_base[pos_s // P] + k) * P + (pos_s % P)
    idxflat = np.zeros(S1 * P, np.int16)
    idxflat[flat_j] = val_s
    maskflat = np.zeros(S1 * P, np.float32)
    maskflat[flat_j] = 1.0
    l1_mask = np.ascontiguousarray(maskflat.reshape(S1, P).T)
    l1_eidx = np.concatenate(
        [_wrap16(idxflat[sb["slot0"] * P:(sb["slot0"] + sb["S"]) * P]) for sb in sbs1],
        axis=1,
    )
    dv1 = np.zeros(npos1, np.int16)
    dv1[:K] = uindex[spec_by_pos]
    l1_didx = np.concatenate(
        [_wrap16(dv1[P * sb["b0"]:P * sb["b1"]]) for sb in sbs1], axis=1
    )
    dp1 = np.zeros(npos1, np.float32)
    dp1[:K] = (deg1[order1] > 0)
    l1_degpos = np.ascontiguousarray(dp1.reshape(nblk1, P).T)

    # ---- layer 2 structure: special dsts only ----
    npos = ((NPC + P - 1) // P) * P
    deg = np.bincount(d2, minlength=N)
    spmask = rowmap[s2] > 0
    sp_d = d2[spmask]
    sp_r = rowmap[s2[spmask]].astype(np.int64)   # table row (1..K) per special edge

    core_raw = []
    for c in range(NCORES):
        lo = c * NPC
        sel = (sp_d >= lo) & (sp_d < lo + NPC)
        e_d = sp_d[sel] - lo
        e_r = sp_r[sel]
        sd, kcnt = np.unique(e_d, return_counts=True)
        z = np.nonzero(deg[lo:lo + NPC] == 0)[0]
        z = np.setdiff1d(z, sd, assume_unique=True)
        nodes = np.concatenate([sd, z])
        kf = np.concatenate([kcnt, np.zeros(len(z), np.int64)])
        o = np.argsort(-kf, kind="stable")
        nodes, kf = nodes[o], kf[o]
        cv = (deg[lo + nodes] - kf).astype(np.float32)
        core_raw.append({"nodes": nodes, "kf": kf, "cv": cv,
                         "e_d": e_d, "e_r": e_r})

    nsd_max = max(len(cd["nodes"]) for cd in core_raw)
    NSB = max(1, (nsd_max + P - 1) // P)
    assert NSB * P <= npos - P, "too many special dsts for this layout"
    Lb = []
    for b in range(NSB):
        mx = 1
        for cd in core_raw:
            kk = cd["kf"][b * P:(b + 1) * P]
            if len(kk):
                mx = max(mx, int(kk.max()))
        Lb.append(mx)
    slot_off = np.concatenate([[0], np.cumsum(Lb)])[:-1].astype(np.int64)
    S2 = int(sum(Lb))
    groups2 = []
    b = 0
    while b < NSB:
        b0 = b
        while b < NSB and Lb[b] == Lb[b0]:
            b += 1
        groups2.append({"b0": b0, "B": b - b0, "L": Lb[b0],
                        "slot_off": int(slot_off[b0])})
    SP = NSB * P
    Wst = (npos - SP) // P

    rng = np.random.default_rng(12345)
    cores = []
    for c, cd in enumerate(core_raw):
        nodes, kf, cv = cd["nodes"], cd["kf"], cd["cv"]
        nsd = len(nodes)
        rank_local = np.full(NPC, -1, np.int64)
        rank_local[nodes] = np.arange(nsd)
        i_e = rank_local[cd["e_d"]]
        pe2 = np.argsort(i_e, kind="stable")
        i_s = i_e[pe2]
        r_s = cd["e_r"][pe2]
        start = np.searchsorted(i_s, np.arange(nsd))
        kk = np.arange(len(i_s)) - start[i_s]
        blk = i_s // P
        prt = i_s % P
        flat_j = (slot_off[blk] + kk) * P + prt
        eflat = rng.integers(K + 1, VTAB, S2 * P).astype(np.int16)
        eflat[flat_j] = r_s.astype(np.int16)
        maskflat = np.zeros(S2 * P, np.float32)
        maskflat[flat_j] = 1.0
        mask = np.ascontiguousarray(maskflat.reshape(S2, P).T)
        # didx slots: the dst node's own table row (0 -> use a default replica)
        dflat = rng.integers(K + 1, VTAB, NSB * P).astype(np.int16)
        dvals = rowmap[c * NPC + nodes]
        dz = dvals == 0
        dvals = dvals.copy()
        dvals[dz] = rng.integers(K + 1, VTAB, int(dz.sum())).astype(np.int16)
        dflat[:nsd] = dvals
        eidx = _wrap16(np.concatenate([eflat, dflat]))
        cvals = np.ones(NSB * P, np.float32)
        cvals[:nsd] = cv
        cdef = np.ascontiguousarray(cvals.reshape(NSB, P).T)
        # output rows: node at sorted index i -> grid (b=i//P, p=i%P) -> row
        # p*NSB + b; pad positions produce r1 -> assign filler default nodes.
        others = np.setdiff1d(np.arange(NPC), nodes)
        assert len(others) >= SP - nsd
        nodes_ext = np.concatenate([nodes, others[:SP - nsd]])
        order_sp = nodes_ext.reshape(NSB, P).T.reshape(-1)
        order = np.concatenate([order_sp, others[SP - nsd:]])
        cores.append({"eidx": eidx, "mask": mask, "cdef": cdef, "order": order})

    meta = {
        "K": K, "K1pos": K1pos, "nblk1": nblk1, "nU": nU, "nUt": nUt,
        "L1": L1, "sbs1": sbs1, "groups1": groups1, "S1": S1,
        "NSB": NSB, "S2": S2, "Lb2": Lb, "groups2": groups2,
        "npos": npos, "SP": SP, "Wst": Wst,
    }
    l1 = {"uidx": uidx, "l1_eidx": l1_eidx, "l1_didx": l1_didx,
          "l1_mask": l1_mask, "l1_degpos": l1_degpos}
    return meta, l1, cores


def _emit_group(nc, gw, Gap, mask_ap, adst_ap, degpos_ap, B, L):
    """Layer-1 segment softmax + weighted sum (B blocks, padded degree L)."""
    BL = B * L
    asrc = Gap[:, :, 64:65].rearrange("p s o -> p (s o)")        # [128, BL]
    s_t = gw.tile([P, B, L], F32, tag="s_t")
    nc.vector.tensor_tensor(s_t[:], asrc, adst_ap.to_broadcast((P, B, L)),
                            op=OP.add)
    u_t = gw.tile([P, B, L], F32, tag="u_t")
    nc.vector.scalar_tensor_tensor(u_t[:], s_t[:], NEG_SLOPE, s_t[:],
                                   op0=OP.mult, op1=OP.max)
    e2_t = gw.tile([P, B, L], F32, tag="e2_t")
    nc.vector.scalar_tensor_tensor(e2_t[:], u_t[:], BIG, mask_ap,
                                   op0=OP.add, op1=OP.mult)
    mneg = gw.tile([P, B], F32, tag="mneg")
    nc.vector.tensor_reduce(mneg[:], e2_t[:], axis=mybir.AxisListType.X,
                            op=OP.max, negate=True)
    d_t = gw.tile([P, B, L], F32, tag="d_t")
    nc.vector.tensor_tensor(d_t[:], e2_t[:], mneg[:].to_broadcast((P, B, L)),
                            op=OP.add)
    ex_t = gw.tile([P, B, L], F32, tag="ex_t")
    nc.scalar.activation(ex_t[:], d_t[:], AF.Exp)
    ssum = gw.tile([P, B], F32, tag="ssum")
    nc.vector.tensor_reduce(ssum[:], ex_t[:], axis=mybir.AxisListType.X,
                            op=OP.add)
    sp = gw.tile([P, B], F32, tag="sp")
    nc.vector.tensor_scalar_add(sp[:], ssum[:], EPS)
    rs = gw.tile([P, B], F32, tag="rs")
    nc.vector.reciprocal(rs[:], sp[:])
    rsd = gw.tile([P, B], F32, tag="rsd")
    nc.vector.tensor_tensor(rsd[:], rs[:], degpos_ap, op=OP.mult)
    alpha = gw.tile([P, B, L], F32, tag="alpha")
    nc.vector.tensor_tensor(alpha[:], ex_t[:], rsd[:].to_broadcast((P, B, L)),
                            op=OP.mult)
    wr = gw.tile([P, BL, D], F32, tag="wr")
    nc.vector.tensor_tensor(wr[:], Gap[:, :, 0:D],
                            alpha[:].rearrange("p b l -> p (b l)")
                            .to_broadcast((P, BL, D)), op=OP.mult)
    msg = gw.tile([P, B, D], F32, tag="msg")
    nc.vector.tensor_reduce(msg[:], wr[:].rearrange("p (b l) f -> p b f l", b=B),
                            axis=mybir.AxisListType.X, op=OP.add)
    return msg


def _emit_special(nc, gw, Gap, mask_ap, adstB_ap, udB_ap, cdef_ap, xdefB_ap,
                  B, L):
    """Layer-2 segment softmax over special edges with an analytic default
    pseudo-term of multiplicity c_v.  Returns msg [128, B, 64] in x1-space."""
    BL = B * L
    asrc = Gap[:, :, 64:65].rearrange("p s o -> p (s o)")        # [128, BL]
    s_t = gw.tile([P, B, L], F32, tag="s_t")
    nc.vector.tensor_tensor(s_t[:], asrc, adstB_ap.to_broadcast((P, B, L)),
                            op=OP.add)
    u_t = gw.tile([P, B, L], F32, tag="u_t")
    nc.vector.scalar_tensor_tensor(u_t[:], s_t[:], NEG_SLOPE, s_t[:],
                                   op0=OP.mult, op1=OP.max)
    e2_t = gw.tile([P, B, L], F32, tag="e2_t")
    nc.vector.scalar_tensor_tensor(e2_t[:], u_t[:], BIG, mask_ap,
                                   op0=OP.add, op1=OP.mult)
    mx = gw.tile([P, B], F32, tag="mx")
    nc.vector.tensor_reduce(mx[:], e2_t[:], axis=mybir.AxisListType.X,
                            op=OP.max)
    mx2 = gw.tile([P, B], F32, tag="mx2")
    nc.vector.tensor_tensor(mx2[:], mx[:], udB_ap, op=OP.max)
    d_t = gw.tile([P, B, L], F32, tag="d_t")
    nc.vector.tensor_tensor(d_t[:], e2_t[:], mx2[:].to_broadcast((P, B, L)),
                            op=OP.subtract)
    ex_t = gw.tile([P, B, L], F32, tag="ex_t")
    nc.scalar.activation(ex_t[:], d_t[:], AF.Exp)
    dd = gw.tile([P, B], F32, tag="dd")
    nc.vector.tensor_tensor(dd[:], udB_ap, mx2[:], op=OP.subtract)
    exd = gw.tile([P, B], F32, tag="exd")
    nc.scalar.activation(exd[:], dd[:], AF.Exp)
    wd = gw.tile([P, B], F32, tag="wd")
    nc.vector.tensor_tensor(wd[:], cdef_ap, exd[:], op=OP.mult)
    ssum = gw.tile([P, B], F32, tag="ssum")
    nc.vector.tensor_reduce(ssum[:], ex_t[:], axis=mybir.AxisListType.X,
                            op=OP.add)
    tot = gw.tile([P, B], F32, tag="tot")
    nc.vector.scalar_tensor_tensor(tot[:], wd[:], EPS, ssum[:],
                                   op0=OP.add, op1=OP.add)
    rs = gw.tile([P, B], F32, tag="rs")
    nc.vector.reciprocal(rs[:], tot[:])
    alpha = gw.tile([P, B, L], F32, tag="alpha")
    nc.vector.tensor_tensor(alpha[:], ex_t[:], rs[:].to_broadcast((P, B, L)),
                            op=OP.mult)
    wdef = gw.tile([P, B], F32, tag="wdef")
    nc.vector.tensor_tensor(wdef[:], wd[:], rs[:], op=OP.mult)
    wr = gw.tile([P, BL, D], F32, tag="wr")
    nc.vector.tensor_tensor(wr[:], Gap[:, :, 0:D],
                            alpha[:].rearrange("p b l -> p (b l)")
                            .to_broadcast((P, BL, D)), op=OP.mult)
    msg = gw.tile([P, B, D], F32, tag="msg")
    nc.vector.tensor_reduce(msg[:], wr[:].rearrange("p (b l) f -> p b f l", b=B),
                            axis=mybir.AxisListType.X, op=OP.add)
    wx = gw.tile([P, B, D], F32, tag="wx")
    nc.vector.tensor_tensor(wx[:], xdefB_ap,
                            wdef[:].to_broadcast((P, B, D)), op=OP.mult)
    msgf = gw.tile([P, B, D], F32, tag="msgf")
    nc.vector.tensor_tensor(msgf[:], msg[:], wx[:], op=OP.add)
    return msgf


def build(meta, repeat=1):
    """Build the SPMD Bass program (common across cores)."""
    K = meta["K"]
    nblk1, nUt = meta["nblk1"], meta["nUt"]
    S1, sbs1, groups1 = meta["S1"], meta["sbs1"], meta["groups1"]
    NSB, S2, groups2 = meta["NSB"], meta["S2"], meta["groups2"]
    npos, SP, Wst = meta["npos"], meta["SP"], meta["Wst"]

    nc = bacc.Bacc("TRN2", target_bir_lowering=False, debug=False,
                   num_devices=NCORES)
    dt = nc.dram_tensor
    x_in = dt("x_in", [N, D], F32, kind="ExternalInput").ap()
    W1_in = dt("W1_in", [D, D], F32, kind="ExternalInput").ap()
    W1T_in = dt("W1T_in", [D, D], F32, kind="ExternalInput").ap()
    W2_in = dt("W2_in", [D, D], F32, kind="ExternalInput").ap()
    W2T_in = dt("W2T_in", [D, D], F32, kind="ExternalInput").ap()
    av1_in = dt("av1_in", [D, 2], F32, kind="ExternalInput").ap()
    av2_in = dt("av2_in", [D, 2], F32, kind="ExternalInput").ap()
    b1row_in = dt("b1row_in", [1, D], F32, kind="ExternalInput").ap()
    b2row_in = dt("b2row_in", [1, D], F32, kind="ExternalInput").ap()
    b1col_in = dt("b1col_in", [D, 1], F32, kind="ExternalInput").ap()
    uidx_in = dt("uidx_in", [P, nUt], I32, kind="ExternalInput").ap()
    l1_eidx_in = dt("l1_eidx_in", [P, 8 * S1], I16, kind="ExternalInput").ap()
    l1_didx_in = dt("l1_didx_in", [P, 8 * nblk1], I16, kind="ExternalInput").ap()
    l1_mask_in = dt("l1_mask_in", [P, S1], F32, kind="ExternalInput").ap()
    l1_degpos_in = dt("l1_degpos_in", [P, nblk1], F32, kind="ExternalInput").ap()
    eidx_in = dt("eidx_in", [P, 8 * (S2 + NSB)], I16, kind="ExternalInput").ap()
    mask_in = dt("mask_in", [P, S2], F32, kind="ExternalInput").ap()
    cdef_in = dt("cdef_in", [P, NSB], F32, kind="ExternalInput").ap()
    out_t = dt("out", [npos, D], F32, kind="ExternalOutput").ap()

    h1tab = dt("h1tab", [nUt * P, P], F32).ap()
    tab = dt("tab", [VTAB, P], F32).ap()

    with tile.TileContext(nc) as tc, ExitStack() as ctx:
        const = ctx.enter_context(tc.tile_pool(name="const", bufs=1))
        psc_ctx = tc.tile_pool(name="psc", bufs=1, space="PSUM")
        psc = psc_ctx.__enter__()

        ident = const.tile([P, P], F32)
        make_identity(nc, ident[:])

        # ---- weights / augmented matrices ----
        W1s = const.tile([D, D], F32)
        nc.sync.dma_start(W1s[:], W1_in[:])
        W1Ts = const.tile([D, D], F32)
        nc.sync.dma_start(W1Ts[:], W1T_in[:])
        W2s = const.tile([D, D], F32)
        nc.sync.dma_start(W2s[:], W2_in[:])
        W2Ts = const.tile([D, D], F32)
        nc.sync.dma_start(W2Ts[:], W2T_in[:])
        av1s = const.tile([D, 2], F32)
        nc.sync.dma_start(av1s[:], av1_in[:])
        av2s = const.tile([D, 2], F32)
        nc.sync.dma_start(av2s[:], av2_in[:])
        b1cols = const.tile([D, 1], F32)
        nc.sync.dma_start(b1cols[:], b1col_in[:])
        b2rows = const.tile([1, D], F32)
        nc.sync.dma_start(b2rows[:], b2row_in[:])

        wt1_p = psc.tile([D, 2], F32, space="PSUM")
        nc.tensor.matmul(wt1_p[:], W1Ts[:], av1s[:], start=True, stop=True)
        wt2_p = psc.tile([D, 2], F32, space="PSUM")
        nc.tensor.matmul(wt2_p[:], W2Ts[:], av2s[:], start=True, stop=True)
        wt2s = const.tile([D, 2], F32)
        nc.vector.tensor_copy(wt2s[:], wt2_p[:])

        W1aug = const.tile([D, D + 2], F32)
        nc.vector.tensor_copy(W1aug[:, 0:D], W1s[:])
        nc.vector.tensor_copy(W1aug[:, D:D + 2], wt1_p[:])

        # SPEC [65, 66] = [[I | wt2s | wt2d]; [b1 | b1.wt2s | b1.wt2d]]
        SPEC = const.tile([D + 1, D + 2], F32)
        nc.vector.tensor_copy(SPEC[0:D, 0:D], ident[0:D, 0:D])
        nc.vector.tensor_copy(SPEC[0:D, D:D + 2], wt2s[:])
        nc.sync.dma_start(SPEC[D:D + 1, 0:D], b1row_in[:])
        b1w_p = psc.tile([1, 2], F32, space="PSUM")
        nc.tensor.matmul(b1w_p[:], b1cols[:], wt2s[:], start=True, stop=True)
        nc.vector.tensor_copy(SPEC[D:D + 1, D:D + 2], b1w_p[:])

        W2OUT = const.tile([D + 1, D], F32)
        nc.vector.tensor_copy(W2OUT[0:D, :], W2s[:])
        nc.sync.dma_start(W2OUT[D:D + 1, :], b2row_in[:])

        # ---- default-row derived constants ----
        # b1aug [1, 66] = [b1 | b1.wt2s | b1.wt2d]
        b1aug = const.tile([1, D + 2], F32)
        nc.sync.dma_start(b1aug[0:1, 0:D], b1row_in[:])
        nc.vector.tensor_copy(b1aug[0:1, D:D + 2], b1w_p[:])
        # r1row [1, 64] = b1 @ W2 + b2
        r1_p = psc.tile([1, D], F32, space="PSUM")
        nc.tensor.matmul(r1_p[:], b1cols[:], W2s[:], start=True, stop=True)
        r1row = const.tile([1, D], F32)
        nc.vector.tensor_tensor(r1row[:], r1_p[:], b2rows[:], op=OP.add)
        # broadcast across partitions via ones matmul
        ones1 = const.tile([1, P], F32)
        nc.vector.memset(ones1[:], 1.0)
        xdef_p = psc.tile([P, D + 2], F32, space="PSUM")
        nc.tensor.matmul(xdef_p[:], ones1[:], b1aug[:], start=True, stop=True)
        xdefbc = const.tile([P, D + 2], F32)
        nc.vector.tensor_copy(xdefbc[:], xdef_p[:])
        r1_pb = psc.tile([P, D], F32, space="PSUM")
        nc.tensor.matmul(r1_pb[:], ones1[:], r1row[:], start=True, stop=True)
        # wide broadcast tile [P, Wst*64] (r1 repeated) built by doubling
        wide = const.tile([P, Wst * D], F32)
        nc.vector.tensor_copy(wide[:, 0:D], r1_pb[:])
        w = 1
        while w < Wst:
            n = min(w, Wst - w)
            nc.vector.tensor_copy(wide[:, w * D:(w + n) * D], wide[:, 0:n * D])
            w += n
        # xdefB [P, NSB*64] (b1 row repeated per block)
        xdefB = const.tile([P, NSB * D], F32)
        nc.vector.tensor_copy(xdefB[:, 0:D], xdefbc[:, 0:D])
        w = 1
        while w < NSB:
            n = min(w, NSB - w)
            nc.vector.tensor_copy(xdefB[:, w * D:(w + n) * D], xdefB[:, 0:n * D])
            w += n

        psc_ctx.__exit__(None, None, None)

        # ---- layer 1: build h1 table for the U endpoint nodes ----
        uidx_s = const.tile([P, nUt], I32)
        nc.sync.dma_start(uidx_s[:], uidx_in[:])
        with tc.tile_pool(name="l1u", bufs=2) as l1u, \
             tc.tile_pool(name="l1up", bufs=2, space="PSUM") as l1up:
            for t in range(nUt):
                xU = l1u.tile([P, D], F32, tag="xU")
                nc.gpsimd.indirect_dma_start(
                    out=xU[:], out_offset=None, in_=x_in[:, :],
                    in_offset=bass.IndirectOffsetOnAxis(ap=uidx_s[:, t:t + 1], axis=0))
                xT_p = l1up.tile([D, P], F32, space="PSUM", tag="xT")
                nc.tensor.transpose(xT_p[:], xU[:], ident[:])
                xT_s = l1u.tile([D, P], F32, tag="xTs")
                nc.vector.tensor_copy(xT_s[:], xT_p[:])
                h_p = l1up.tile([P, D + 2], F32, space="PSUM", tag="h_p")
                nc.tensor.matmul(h_p[:], xT_s[:], W1aug[:], start=True, stop=True)
                h_s = l1u.tile([P, P], F32, tag="h_s")
                nc.scalar.copy(h_s[:, 0:D + 2], h_p[:])
                nc.vector.memset(h_s[:, D + 2:P], 0.0)
                nc.sync.dma_start(h1tab[t * P:(t + 1) * P, :], h_s[:])

        # ---- layer 1 conv -> write table rows ----
        l1_eidx_s = const.tile([P, 8 * S1], I16)
        nc.sync.dma_start(l1_eidx_s[:], l1_eidx_in[:])
        l1_didx_s = const.tile([P, 8 * nblk1], I16)
        nc.sync.dma_start(l1_didx_s[:], l1_didx_in[:])
        l1_mask_s = const.tile([P, S1], F32)
        nc.sync.dma_start(l1_mask_s[:], l1_mask_in[:])
        l1_degpos_s = const.tile([P, nblk1], F32)
        nc.sync.dma_start(l1_degpos_s[:], l1_degpos_in[:])

        with tc.tile_pool(name="l1w", bufs=2) as l1w, \
             tc.tile_pool(name="l1p", bufs=2, space="PSUM") as l1p:
            dr1 = l1w.tile([P, nblk1, P], F32, tag="dr1")
            nc.gpsimd.dma_gather(dr1[:], h1tab[:, :], l1_didx_s[:],
                                 nblk1 * P, nblk1 * P, P, single_packet=False)
            adst1 = l1w.tile([P, nblk1], F32, tag="adst1")
            nc.scalar.activation(adst1[:],
                                 dr1[:, 0:nblk1, 65:66].rearrange("p b o -> p (b o)"),
                                 AF.Identity)
            for sb_i, sb in enumerate(sbs1):
                G1 = l1w.tile([P, sb["S"], P], F32, tag="G1")
                nc.gpsimd.dma_gather(
                    G1[:], h1tab[:, :],
                    l1_eidx_s[:, 8 * sb["slot0"]:8 * (sb["slot0"] + sb["S"])],
                    sb["S"] * P, sb["S"] * P, P, single_packet=False)
                for g in [g for g in groups1 if g["sb"] == sb_i]:
                    B, L, off = g["B"], g["L"], g["slot_off"]
                    sl0 = sb["slot0"] + off
                    msg = _emit_group(
                        nc, l1w, G1[:, off:off + B * L, :],
                        l1_mask_s[:, sl0:sl0 + B * L],
                        adst1[:, g["b0"]:g["b0"] + B],
                        l1_degpos_s[:, g["b0"]:g["b0"] + B], B, L)
                    for j in range(B):
                        b = g["b0"] + j
                        mT_p = l1p.tile([D, P], F32, space="PSUM", tag="mT")
                        nc.tensor.transpose(mT_p[:], msg[:, j, :], ident[:])
                        mT_s = l1w.tile([D + 1, P], F32, tag="mTs")
                        nc.vector.tensor_copy(mT_s[0:D, :], mT_p[:])
                        nc.vector.memset(mT_s[D:D + 1, :], 1.0)
                        row_p = l1p.tile([P, D + 2], F32, space="PSUM", tag="rowp")
                        nc.tensor.matmul(row_p[:], mT_s[:], SPEC[:],
                                         start=True, stop=True)
                        row_s = l1w.tile([P, P], F32, tag="rows")
                        nc.scalar.copy(row_s[:, 0:D + 2], row_p[:])
                        nc.vector.memset(row_s[:, D + 2:P], 0.0)
                        nrows = min(P, K - b * P)
                        if nrows > 0:
                            nc.sync.dma_start(
                                tab[1 + b * P:1 + b * P + nrows, :],
                                row_s[0:nrows, :])
                        if b == K // P:   # default row from the pad position K
                            q = K % P
                            nc.sync.dma_start(tab[0:1, :], row_s[q:q + 1, :])
                            # replicate the default row over rows K+1..VTAB-1
                            # (spreads default-row gather traffic across HBM)
                            zidx = l1w.tile([P, 8], I16, tag="zidx")
                            nc.vector.memset(zidx[:], 0)
                            defbc = l1w.tile([P, 1, P], F32, tag="defbc")
                            nc.gpsimd.dma_gather(defbc[:], tab[:, :], zidx[:],
                                                 P, P, P, single_packet=False)
                            r0 = K + 1
                            while r0 < VTAB:
                                cnt = min(P, VTAB - r0)
                                nc.sync.dma_start(tab[r0:r0 + cnt, :],
                                                  defbc[0:cnt, 0, :])
                                r0 += cnt

        # ---- layer 2 ----
        eidx_s = const.tile([P, 8 * (S2 + NSB)], I16)
        nc.sync.dma_start(eidx_s[:], eidx_in[:])
        mask_s = const.tile([P, S2], F32)
        nc.sync.dma_start(mask_s[:], mask_in[:])
        cdef_s = const.tile([P, NSB], F32)
        nc.sync.dma_start(cdef_s[:], cdef_in[:])

        with tc.tile_pool(name="gw", bufs=2) as gw, \
             tc.tile_pool(name="blk", bufs=3) as blk, \
             tc.tile_pool(name="psb", bufs=3, space="PSUM") as psb:
            for _rep in range(repeat):
                Gall = gw.tile([P, S2 + NSB, P], F32, tag="Gall")
                nc.gpsimd.dma_gather(Gall[:], tab[:, :], eidx_s[:],
                                     (S2 + NSB) * P, (S2 + NSB) * P, P,
                                     single_packet=False)
                adst = gw.tile([P, NSB], F32, tag="adst")
                nc.scalar.activation(
                    adst[:],
                    Gall[:, S2:S2 + NSB, 65:66].rearrange("p b o -> p (b o)"),
                    AF.Identity)
                # default-mass score: udB = leaky(asrc_def + adst) + BIG
                zt = gw.tile([P, NSB], F32, tag="zt")
                nc.vector.tensor_tensor(
                    zt[:], adst[:],
                    xdefbc[:, D:D + 1].to_broadcast((P, NSB)), op=OP.add)
                ut = gw.tile([P, NSB], F32, tag="ut")
                nc.vector.scalar_tensor_tensor(ut[:], zt[:], NEG_SLOPE, zt[:],
                                               op0=OP.mult, op1=OP.max)
                udB = gw.tile([P, NSB], F32, tag="udB")
                nc.vector.tensor_scalar_add(udB[:], ut[:], BIG)

                osp = blk.tile([P, NSB, D], F32, tag="osp")
                for g in groups2:
                    B, L, off = g["B"], g["L"], g["slot_off"]
                    b0 = g["b0"]
                    msgf = _emit_special(
                        nc, gw, Gall[:, off:off + B * L, :],
                        mask_s[:, off:off + B * L],
                        adst[:, b0:b0 + B], udB[:, b0:b0 + B],
                        cdef_s[:, b0:b0 + B],
                        xdefB[:].rearrange("p (b f) -> p b f", f=D)
                        [:, b0:b0 + B, :],
                        B, L)
                    for j in range(B):
                        b = b0 + j
                        mT_p = psb.tile([D, P], F32, space="PSUM", tag="mT")
                        nc.tensor.transpose(mT_p[:], msgf[:, j, :], ident[:])
                        mT_s = blk.tile([D + 1, P], F32, tag="mTs")
                        nc.vector.tensor_copy(mT_s[0:D, :], mT_p[:])
                        nc.vector.memset(mT_s[D:D + 1, :], 1.0)
                        o_p = psb.tile([P, D], F32, space="PSUM", tag="op")
                        nc.tensor.matmul(o_p[:], mT_s[:], W2OUT[:],
                                         start=True, stop=True)
                        nc.scalar.copy(osp[:, b, :], o_p[:])
                # special rows: row p*NSB+b <- osp[p, b]
                nc.sync.dma_start(
                    out_t[0:SP, :].rearrange("(p b) f -> p (b f)", p=P),
                    osp[:].rearrange("p b f -> p (b f)"))
                # broadcast r1 over all remaining rows
                nc.sync.dma_start(
                    out_t[SP:npos, :].rearrange("(p w) f -> p (w f)", p=P),
                    wide[:])

    nc.compile()
    return nc


def make_in_maps(inputs, meta, l1, cores):
    x = np.ascontiguousarray(np.asarray(inputs["x"], dtype=np.float32))
    W1 = np.asarray(inputs["W1"], dtype=np.float32)
    W2 = np.asarray(inputs["W2"], dtype=np.float32)
    base = {
        "x_in": x,
        "W1_in": np.ascontiguousarray(W1),
        "W1T_in": np.ascontiguousarray(W1.T),
        "W2_in": np.ascontiguousarray(W2),
        "W2T_in": np.ascontiguousarray(W2.T),
        "av1_in": np.ascontiguousarray(np.stack(
            [np.asarray(inputs["a_src1"]), np.asarray(inputs["a_dst1"])],
            axis=1).astype(np.float32)),
        "av2_in": np.ascontiguousarray(np.stack(
            [np.asarray(inputs["a_src2"]), np.asarray(inputs["a_dst2"])],
            axis=1).astype(np.float32)),
        "b1row_in": np.asarray(inputs["b1"], dtype=np.float32).reshape(1, D),
        "b2row_in": np.asarray(inputs["b2"], dtype=np.float32).reshape(1, D),
        "b1col_in": np.asarray(inputs["b1"], dtype=np.float32).reshape(D, 1),
        "uidx_in": l1["uidx"],
        "l1_eidx_in": l1["l1_eidx"],
        "l1_didx_in": l1["l1_didx"],
        "l1_mask_in": l1["l1_mask"],
        "l1_degpos_in": l1["l1_degpos"],
    }
    in_maps = []
    for c in range(NCORES):
        m = dict(base)
        m["eidx_in"] = cores[c]["eidx"]
        m["mask_in"] = cores[c]["mask"]
        m["cdef_in"] = cores[c]["cdef"]
        in_maps.append(m)
    return in_maps


def unshard(results, cores):
    out = np.empty((N, D), np.float32)
    for c in range(NCORES):
        oc = results[c]["out"]
        order = cores[c]["order"]
        out[c * NPC + order] = oc[:NPC]
    return out


def kernel(**inputs):
    meta, l1, cores = prep(inputs)
    nc = build(meta, repeat=1)
    in_maps = make_in_maps(inputs, meta, l1, cores)
    res = run_bass_kernel_spmd(nc, in_maps, core_ids=list(range(NCORES)))
    return unshard(res.results, cores)


# revision 49
# speedup vs baseline: 351.2854x; 351.2854x over previous
"""GAT 2-layer encoder on 8 Trainium2 NeuronCores.

Reference computation: layer 1 = GAT conv over edge_index[:, :500] (weights W1),
layer 2 = GAT conv over edge_index[:, 500:] (weights W2).

Strategy (constant-default decomposition):
  - Layer-1 output x1 differs from b1 only on the K<=500 distinct dsts of the
    first 500 edges ("specials").  In layer 2, every dst whose in-edges all
    come from default srcs sees identical attention scores -> uniform softmax
    -> output row r1 = b1@W2 + b2, independent of adst.  Only dsts with >=1
    special in-edge (~E*K/N ~ 8000 nodes) need real computation; deg-0 dsts
    get b2 (handled as specials with all-zero weights, via the +EPS in the
    reference's segment softmax).
  - Sharding: dst-range partition across 8 cores.  Layer 1 + the K+1-row
    gather table (512B rows [x1 | x1.(W2 a_src2) | x1.(W2 a_dst2) | pad]) are
    replicated on every core (tiny).
  - Per core and per iteration: one dma_gather for the ~1k special-edge rows
    plus one 512B row per special dst (for adst), a segment softmax with an
    analytic "default mass" pseudo-term (weight c_v = #default in-edges), one
    matmul per 128-dst block, then two big partition-striped DMA writes:
    the special block rows and the broadcast of r1 over all remaining rows.
"""

import sys

sys.path.insert(0, "/opt/trn_rl_repo")

from contextlib import ExitStack

import numpy as np

import concourse.bacc as bacc
import concourse.bass as bass
import concourse.mybir as mybir
import concourse.tile as tile
from concourse.bass_utils import run_bass_kernel_spmd
from concourse.masks import make_identity

F32 = mybir.dt.float32
I16 = mybir.dt.int16
I32 = mybir.dt.int32
AF = mybir.ActivationFunctionType
OP = mybir.AluOpType

N = 100000
D = 64
NCORES = 8
NPC = N // NCORES          # dst nodes per core
P = 128
NSPLIT = 500               # first 500 edges -> layer 1
SMAX = 80                  # max edge-slots per superblock (layer-1 SBUF budget)
NEG_SLOPE = 0.2
EPS = 1e-16
BIG = 200.0                # score shift so padded slots underflow exp to 0.0
VTAB = 1024                # gather table rows (specials + default replicas)


def _wrap16(flat):
    """int16 stream [n] (n%16==0) -> dma_gather idx tile [128, n//16]."""
    w = flat.reshape(-1, 16).T
    return np.ascontiguousarray(np.tile(w, (8, 1)).astype(np.int16))


def _grid(deg_sorted_max, npos):
    """Layer-1 block structure from the descending degree profile."""
    nblocks = npos // P
    L = [max(int(deg_sorted_max[b * P]), 1) for b in range(nblocks)]
    sbs = []
    b = 0
    while b < nblocks:
        s = 0
        b0 = b
        while b < nblocks and (b - b0) < 16 and s + L[b] <= max(SMAX, L[b0]):
            s += L[b]
            b += 1
        sbs.append({"b0": b0, "b1": b, "S": s})
    slot0 = 0
    for sb in sbs:
        sb["slot0"] = slot0
        slot0 += sb["S"]
    groups = []
    for si, sb in enumerate(sbs):
        off = 0
        b = sb["b0"]
        while b < sb["b1"]:
            b0 = b
            while b < sb["b1"] and L[b] == L[b0]:
                b += 1
            groups.append({"sb": si, "b0": b0, "B": b - b0, "L": L[b0], "slot_off": off})
            off += (b - b0) * L[b0]
    return L, sbs, groups


def prep(inputs):
    """Host-side index prep (pure index computation, no feature values)."""
    ei = np.asarray(inputs["edge_index"])
    src = ei[0].astype(np.int64)
    dst = ei[1].astype(np.int64)
    s1, d1 = src[:NSPLIT], dst[:NSPLIT]
    s2, d2 = src[NSPLIT:], dst[NSPLIT:]

    # ---- layer 1 structure ----
    specials, deg1 = np.unique(d1, return_counts=True)
    K = len(specials)
    order1 = np.argsort(-deg1, kind="stable")
    spec_by_pos = specials[order1]          # grid position q -> node, table row q+1
    rowmap = np.zeros(N, np.int16)
    rowmap[spec_by_pos] = np.arange(1, K + 1)
    K1pos = K + 1                            # one guaranteed pad slot (default row)
    nblk1 = (K1pos + P - 1) // P
    npos1 = nblk1 * P

    U = np.unique(np.concatenate([s1, d1]))
    nU = len(U)
    nUt = (nU + P - 1) // P
    upad = np.zeros(nUt * P, np.int64)
    upad[:nU] = U
    uindex = np.zeros(N, np.int64)
    uindex[U] = np.arange(nU)

    # layer-1 edge grid (dst -> grid position via rank over specials)
    rank1 = np.empty(K, np.int64)
    rank1[order1] = np.arange(K)
    d1pos = rank1[np.searchsorted(specials, d1)]
    deg1_sorted = np.zeros(npos1, np.int64)
    deg1_sorted[:K] = deg1[order1]
    L1, sbs1, groups1 = _grid(deg1_sorted, npos1)
    S1 = int(sum(L1))
    pe = np.argsort(d1pos, kind="stable")
    pos_s = d1pos[pe]
    val_s = uindex[s1[pe]].astype(np.int16)
    start_of_pos = np.searchsorted(pos_s, np.arange(npos1))
    k = np.arange(len(pos_s)) - start_of_pos[pos_s]
    slot_base = np.concatenate([[0], np.cumsum(L1)])[:-1]
    flat_j = (slot_base[pos_s // P] + k) * P + (pos_s % P)
    idxflat = np.zeros(S1 * P, np.int16)
    idxflat[flat_j] = val_s
    maskflat = np.zeros(S1 * P, np.float32)
    maskflat[flat_j] = 1.0
    l1_mask = np.ascontiguousarray(maskflat.reshape(S1, P).T)
    l1_eidx = np.concatenate(
        [_wrap16(idxflat[sb["slot0"] * P:(sb["slot0"] + sb["S"]) * P]) for sb in sbs1],
        axis=1,
    )
    dv1 = np.zeros(npos1, np.int16)
    dv1[:K] = uindex[spec_by_pos]
    l1_didx = np.concatenate(
        [_wrap16(dv1[P * sb["b0"]:P * sb["b1"]]) for sb in sbs1], axis=1
    )
    dp1 = np.zeros(npos1, np.float32)
    dp1[:K] = (deg1[order1] > 0)
    l1_degpos = np.ascontiguousarray(dp1.reshape(nblk1, P).T)

    # ---- layer 2 structure: special dsts only ----
    npos = ((NPC + P - 1) // P) * P
    deg = np.bincount(d2, minlength=N)
    spmask = rowmap[s2] > 0
    sp_d = d2[spmask]
    sp_r = rowmap[s2[spmask]].astype(np.int64)   # table row (1..K) per special edge

    core_raw = []
    for c in range(NCORES):
        lo = c * NPC
        sel = (sp_d >= lo) & (sp_d < lo + NPC)
        e_d = sp_d[sel] - lo
        e_r = sp_r[sel]
        sd, kcnt = np.unique(e_d, return_counts=True)
        z = np.nonzero(deg[lo:lo + NPC] == 0)[0]
        z = np.setdiff1d(z, sd, assume_unique=True)
        nodes = np.concatenate([sd, z])
        kf = np.concatenate([kcnt, np.zeros(len(z), np.int64)])
        o = np.argsort(-kf, kind="stable")
        nodes, kf = nodes[o], kf[o]
        cv = (deg[lo + nodes] - kf).astype(np.float32)
        core_raw.append({"nodes": nodes, "kf": kf, "cv": cv,
                         "e_d": e_d, "e_r": e_r})

    nsd_max = max(len(cd["nodes"]) for cd in core_raw)
    NSB = max(1, (nsd_max + P - 1) // P)
    assert NSB * P <= npos - P, "too many special dsts for this layout"
    Lb = []
    for b in range(NSB):
        mx = 1
        for cd in core_raw:
            kk = cd["kf"][b * P:(b + 1) * P]
            if len(kk):
                mx = max(mx, int(kk.max()))
        Lb.append(mx)
    slot_off = np.concatenate([[0], np.cumsum(Lb)])[:-1].astype(np.int64)
    S2 = int(sum(Lb))
    groups2 = []
    b = 0
    while b < NSB:
        b0 = b
        while b < NSB and Lb[b] == Lb[b0]:
            b += 1
        groups2.append({"b0": b0, "B": b - b0, "L": Lb[b0],
                        "slot_off": int(slot_off[b0])})
    SP = NSB * P
    Wst = (npos - SP) // P
    J = npos // P              # output rows per partition (row r = p*J + j)

    # adst fixups: special dsts that are themselves layer-1 specials need a
    # gathered adst; everyone else uses the analytic default.  Pack fixups
    # into gather slot columns (distinct partitions per column).
    rng = np.random.default_rng(12345)
    core_fix = []
    NCOL = 1
    for c, cd in enumerate(core_raw):
        nodes = cd["nodes"]
        rows_d = rowmap[c * NPC + nodes]
        fi = np.nonzero(rows_d > 0)[0]          # sorted index i of fix nodes
        prt = fi % P
        colcnt = np.bincount(prt, minlength=P)
        ncol = int(colcnt.max()) if len(fi) else 0
        NCOL = max(NCOL, ncol)
        core_fix.append((fi, rows_d[fi]))

    cores = []
    for c, cd in enumerate(core_raw):
        nodes, kf, cv = cd["nodes"], cd["kf"], cd["cv"]
        nsd = len(nodes)
        rank_local = np.full(NPC, -1, np.int64)
        rank_local[nodes] = np.arange(nsd)
        i_e = rank_local[cd["e_d"]]
        pe2 = np.argsort(i_e, kind="stable")
        i_s = i_e[pe2]
        r_s = cd["e_r"][pe2]
        start = np.searchsorted(i_s, np.arange(nsd))
        kk = np.arange(len(i_s)) - start[i_s]
        blk = i_s // P
        prt = i_s % P
        flat_j = (slot_off[blk] + kk) * P + prt
        eflat = rng.integers(K + 1, VTAB, S2 * P).astype(np.int16)
        eflat[flat_j] = r_s.astype(np.int16)
        maskflat = np.zeros(S2 * P, np.float32)
        maskflat[flat_j] = 1.0
        mask = np.ascontiguousarray(maskflat.reshape(S2, P).T)
        # fix slots + select masks
        fflat = rng.integers(K + 1, VTAB, NCOL * P).astype(np.int16)
        fsel = np.zeros((NCOL, NSB * P), np.float32)
        fi, frows = core_fix[c]
        used = np.zeros(P, np.int64)
        for i, r in zip(fi, frows):
            p = int(i % P)
            b = int(i // P)
            q = int(used[p])
            used[p] += 1
            fflat[q * P + p] = r
            fsel[q, b * P + p] = 1.0
        allflat = np.concatenate([eflat, fflat])
        eidx = _wrap16(allflat)
        iidx = np.ascontiguousarray(
            allflat.reshape(-1, P).T.astype(np.int32))
        fselT = np.ascontiguousarray(
            fsel.reshape(NCOL, NSB, P).transpose(2, 0, 1).reshape(P, NCOL * NSB))
        kinvT = np.ascontiguousarray(1.0 - fselT)
        cvals = np.ones(NSB * P, np.float32)
        cvals[:nsd] = cv
        cdef = np.ascontiguousarray(cvals.reshape(NSB, P).T)
        # output rows: partition p owns rows [p*J, (p+1)*J); grid position
        # (b, p) -> row p*J + b (b < NSB); rows j >= NSB hold the r1
        # broadcast.  Pad grid positions also compute r1, so every non-special
        # row maps to a default node; npos - NPC leftover rows are unused.
        i_sp = np.arange(nsd)
        special_rows = (i_sp % P) * J + (i_sp // P)
        used = np.zeros(npos, bool)
        used[special_rows] = True
        rest_rows = np.nonzero(~used)[0]
        others = np.setdiff1d(np.arange(NPC), nodes)
        assert len(rest_rows) >= len(others)
        rows_arr = np.concatenate([special_rows, rest_rows[:len(others)]])
        nodes_arr = np.concatenate([nodes, others])
        cores.append({"eidx": eidx, "iidx": iidx, "mask": mask, "cdef": cdef,
                      "rows": rows_arr, "nodes": nodes_arr,
                      "fsel": fselT, "kinv": kinvT})

    meta = {
        "K": K, "K1pos": K1pos, "nblk1": nblk1, "nU": nU, "nUt": nUt,
        "L1": L1, "sbs1": sbs1, "groups1": groups1, "S1": S1,
        "NSB": NSB, "S2": S2, "Lb2": Lb, "groups2": groups2, "NCOL": NCOL,
        "npos": npos, "SP": SP, "Wst": Wst, "J": J, "sbs2": [],
    }
    l1 = {"upad": upad, "l1_eidx": l1_eidx, "l1_didx": l1_didx,
          "l1_mask": l1_mask, "l1_degpos": l1_degpos}
    return meta, l1, cores


def _emit_group(nc, gw, Gap, mask_ap, adst_ap, degpos_ap, B, L):
    """Layer-1 segment softmax + weighted sum (B blocks, padded degree L)."""
    BL = B * L
    asrc = Gap[:, :, 64:65].rearrange("p s o -> p (s o)")        # [128, BL]
    s_t = gw.tile([P, B, L], F32, tag="s_t")
    nc.vector.tensor_tensor(s_t[:], asrc, adst_ap.to_broadcast((P, B, L)),
                            op=OP.add)
    u_t = gw.tile([P, B, L], F32, tag="u_t")
    nc.vector.scalar_tensor_tensor(u_t[:], s_t[:], NEG_SLOPE, s_t[:],
                                   op0=OP.mult, op1=OP.max)
    e2_t = gw.tile([P, B, L], F32, tag="e2_t")
    nc.vector.scalar_tensor_tensor(e2_t[:], u_t[:], BIG, mask_ap,
                                   op0=OP.add, op1=OP.mult)
    mneg = gw.tile([P, B], F32, tag="mneg")
    nc.vector.tensor_reduce(mneg[:], e2_t[:], axis=mybir.AxisListType.X,
                            op=OP.max, negate=True)
    d_t = gw.tile([P, B, L], F32, tag="d_t")
    nc.vector.tensor_tensor(d_t[:], e2_t[:], mneg[:].to_broadcast((P, B, L)),
                            op=OP.add)
    ex_t = gw.tile([P, B, L], F32, tag="ex_t")
    nc.scalar.activation(ex_t[:], d_t[:], AF.Exp)
    ssum = gw.tile([P, B], F32, tag="ssum")
    nc.vector.tensor_reduce(ssum[:], ex_t[:], axis=mybir.AxisListType.X,
                            op=OP.add)
    sp = gw.tile([P, B], F32, tag="sp")
    nc.vector.tensor_scalar_add(sp[:], ssum[:], EPS)
    rs = gw.tile([P, B], F32, tag="rs")
    nc.vector.reciprocal(rs[:], sp[:])
    rsd = gw.tile([P, B], F32, tag="rsd")
    nc.vector.tensor_tensor(rsd[:], rs[:], degpos_ap, op=OP.mult)
    alpha = gw.tile([P, B, L], F32, tag="alpha")
    nc.vector.tensor_tensor(alpha[:], ex_t[:], rsd[:].to_broadcast((P, B, L)),
                            op=OP.mult)
    wr = gw.tile([P, BL, D], F32, tag="wr")
    nc.vector.tensor_tensor(wr[:], Gap[:, :, 0:D],
                            alpha[:].rearrange("p b l -> p (b l)")
                            .to_broadcast((P, BL, D)), op=OP.mult)
    msg = gw.tile([P, B, D], F32, tag="msg")
    nc.vector.tensor_reduce(msg[:], wr[:].rearrange("p (b l) f -> p b f l", b=B),
                            axis=mybir.AxisListType.X, op=OP.add)
    return msg


def _emit_special(nc, gw, Gap, mask_ap, adstB_ap, wd_ap, cdef_ap, xdefB_ap,
                  b2bc_ap, osp_ap, B, L):
    """Layer-2 segment softmax over special edges with an analytic default
    pseudo-term of multiplicity c_v; h2-space rows.  Unstabilized exp
    (scores are O(1); softmax is shift-invariant so this matches the
    reference's stabilized form to fp32 rounding).  Writes out rows
    (msg + wdef*h2_def + b2) into osp_ap [128, B, 64]."""
    BL = B * L
    asrc = Gap[:, :, 64:65].rearrange("p s o -> p (s o)")        # [128, BL]
    s_t = gw.tile([P, B, L], F32, tag="s_t")
    nc.vector.tensor_tensor(s_t[:], asrc, adstB_ap.to_broadcast((P, B, L)),
                            op=OP.add)
    u_t = gw.tile([P, B, L], F32, tag="u_t")
    nc.vector.scalar_tensor_tensor(u_t[:], s_t[:], NEG_SLOPE, s_t[:],
                                   op0=OP.mult, op1=OP.max)
    ex_t = gw.tile([P, B, L], F32, tag="ex_t")
    nc.scalar.activation(ex_t[:], u_t[:], AF.Exp)
    exm = gw.tile([P, B, L], F32, tag="exm")
    nc.vector.tensor_tensor(exm[:], ex_t[:], mask_ap, op=OP.mult)
    if L == 1:
        ssum_ap = exm[:].rearrange("p b l -> p (b l)")
    else:
        ssum = gw.tile([P, B], F32, tag="ssum")
        nc.vector.tensor_reduce(ssum[:], exm[:], axis=mybir.AxisListType.X,
                                op=OP.add)
        ssum_ap = ssum[:]
    tot = gw.tile([P, B], F32, tag="tot")
    nc.vector.scalar_tensor_tensor(tot[:], wd_ap, EPS, ssum_ap,
                                   op0=OP.add, op1=OP.add)
    rs = gw.tile([P, B], F32, tag="rs")
    nc.vector.reciprocal(rs[:], tot[:])
    alpha = gw.tile([P, B, L], F32, tag="alpha")
    nc.vector.tensor_tensor(alpha[:], exm[:], rs[:].to_broadcast((P, B, L)),
                            op=OP.mult)
    wdef = gw.tile([P, B], F32, tag="wdef")
    nc.vector.tensor_tensor(wdef[:], wd_ap, rs[:], op=OP.mult)
    if L == 1:
        msg = gw.tile([P, B, D], F32, tag="msgA")
        nc.vector.tensor_tensor(msg[:], Gap[:, :, 0:D],
                                alpha[:].rearrange("p b l -> p (b l)")
                                .to_broadcast((P, B, D)), op=OP.mult)
    else:
        wr = gw.tile([P, BL, D], F32, tag="wr")
        nc.vector.tensor_tensor(wr[:], Gap[:, :, 0:D],
                                alpha[:].rearrange("p b l -> p (b l)")
                                .to_broadcast((P, BL, D)), op=OP.mult)
        msg = gw.tile([P, B, D], F32, tag="msg")
        nc.vector.tensor_reduce(msg[:],
                                wr[:].rearrange("p (b l) f -> p b f l", b=B),
                                axis=mybir.AxisListType.X, op=OP.add)
    wx = gw.tile([P, B, D], F32, tag="wx")
    nc.vector.tensor_tensor(wx[:], xdefB_ap,
                            wdef[:].to_broadcast((P, B, D)), op=OP.mult)
    msgf = gw.tile([P, B, D], F32, tag="msgf")
    nc.vector.tensor_tensor(msgf[:], msg[:], wx[:], op=OP.add)
    nc.vector.tensor_tensor(osp_ap, msgf[:],
                            b2bc_ap.rearrange("p (b f) -> p b f", b=1)
                            .to_broadcast((P, B, D)), op=OP.add)


def build(meta, repeat=1, parts=("gather", "compute", "osp", "bcast"),
          bufs=2, single_packet=False, use_indirect=False):
    """Build the SPMD Bass program (common across cores)."""
    K = meta["K"]
    nblk1, nUt = meta["nblk1"], meta["nUt"]
    S1, sbs1, groups1 = meta["S1"], meta["sbs1"], meta["groups1"]
    NSB, S2, groups2 = meta["NSB"], meta["S2"], meta["groups2"]
    npos, SP, Wst = meta["npos"], meta["SP"], meta["Wst"]
    NCOL = meta["NCOL"]

    nc = bacc.Bacc("TRN2", target_bir_lowering=False, debug=False,
                   num_devices=NCORES)
    dt = nc.dram_tensor
    xu_in = dt("xu_in", [nUt * P, D], F32, kind="ExternalInput").ap()
    W1_in = dt("W1_in", [D, D], F32, kind="ExternalInput").ap()
    W1T_in = dt("W1T_in", [D, D], F32, kind="ExternalInput").ap()
    W2_in = dt("W2_in", [D, D], F32, kind="ExternalInput").ap()
    W2T_in = dt("W2T_in", [D, D], F32, kind="ExternalInput").ap()
    av1_in = dt("av1_in", [D, 2], F32, kind="ExternalInput").ap()
    av2_in = dt("av2_in", [D, 2], F32, kind="ExternalInput").ap()
    b1row_in = dt("b1row_in", [1, D], F32, kind="ExternalInput").ap()
    b2row_in = dt("b2row_in", [1, D], F32, kind="ExternalInput").ap()
    b1col_in = dt("b1col_in", [D, 1], F32, kind="ExternalInput").ap()
    l1_eidx_in = dt("l1_eidx_in", [P, 8 * S1], I16, kind="ExternalInput").ap()
    l1_didx_in = dt("l1_didx_in", [P, 8 * nblk1], I16, kind="ExternalInput").ap()
    l1_mask_in = dt("l1_mask_in", [P, S1], F32, kind="ExternalInput").ap()
    l1_degpos_in = dt("l1_degpos_in", [P, nblk1], F32, kind="ExternalInput").ap()
    eidx_in = dt("eidx_in", [P, 8 * (S2 + NCOL)], I16, kind="ExternalInput").ap()
    iidx_in = dt("iidx_in", [P, S2 + NCOL], I32, kind="ExternalInput").ap()
    mask_in = dt("mask_in", [P, S2], F32, kind="ExternalInput").ap()
    cdef_in = dt("cdef_in", [P, NSB], F32, kind="ExternalInput").ap()
    fsel_in = dt("fsel_in", [P, NCOL * NSB], F32, kind="ExternalInput").ap()
    kinv_in = dt("kinv_in", [P, NCOL * NSB], F32, kind="ExternalInput").ap()
    out_t = dt("out", [npos, D], F32, kind="ExternalOutput").ap()

    h1tab = dt("h1tab", [nUt * P, P], F32).ap()
    tab = dt("tab", [VTAB, P], F32).ap()

    with tile.TileContext(nc) as tc, ExitStack() as ctx:
        const = ctx.enter_context(tc.tile_pool(name="const", bufs=1))
        psc_ctx = tc.tile_pool(name="psc", bufs=1, space="PSUM")
        psc = psc_ctx.__enter__()

        ident = const.tile([P, P], F32)
        make_identity(nc, ident[:])

        # ---- weights / augmented matrices ----
        W1s = const.tile([D, D], F32)
        nc.sync.dma_start(W1s[:], W1_in[:])
        W1Ts = const.tile([D, D], F32)
        nc.sync.dma_start(W1Ts[:], W1T_in[:])
        W2s = const.tile([D, D], F32)
        nc.sync.dma_start(W2s[:], W2_in[:])
        W2Ts = const.tile([D, D], F32)
        nc.sync.dma_start(W2Ts[:], W2T_in[:])
        av1s = const.tile([D, 2], F32)
        nc.sync.dma_start(av1s[:], av1_in[:])
        av2s = const.tile([D, 2], F32)
        nc.sync.dma_start(av2s[:], av2_in[:])
        b1cols = const.tile([D, 1], F32)
        nc.sync.dma_start(b1cols[:], b1col_in[:])
        b2rows = const.tile([1, D], F32)
        nc.sync.dma_start(b2rows[:], b2row_in[:])

        wt1_p = psc.tile([D, 2], F32, space="PSUM")
        nc.tensor.matmul(wt1_p[:], W1Ts[:], av1s[:], start=True, stop=True)
        wt2_p = psc.tile([D, 2], F32, space="PSUM")
        nc.tensor.matmul(wt2_p[:], W2Ts[:], av2s[:], start=True, stop=True)
        wt2s = const.tile([D, 2], F32)
        nc.vector.tensor_copy(wt2s[:], wt2_p[:])

        W1aug = const.tile([D, D + 2], F32)
        nc.vector.tensor_copy(W1aug[:, 0:D], W1s[:])
        nc.vector.tensor_copy(W1aug[:, D:D + 2], wt1_p[:])

        # W2aug [64, 66] = [W2 | wt2s | wt2d]
        W2aug = const.tile([D, D + 2], F32)
        nc.vector.tensor_copy(W2aug[:, 0:D], W2s[:])
        nc.vector.tensor_copy(W2aug[:, D:D + 2], wt2s[:])
        # SPEC [65, 66] = [[I; b1] @ W2aug] -> table rows live in h2-space:
        # row(x1) = [x1@W2 | x1.wt2s | x1.wt2d]
        AUG = const.tile([D, D + 1], F32)
        nc.vector.tensor_copy(AUG[:, 0:D], ident[0:D, 0:D])
        nc.vector.tensor_copy(AUG[:, D:D + 1], b1cols[:])
        SPEC_p = psc.tile([D + 1, D + 2], F32, space="PSUM")
        nc.tensor.matmul(SPEC_p[:], AUG[:], W2aug[:], start=True, stop=True)
        SPEC = const.tile([D + 1, D + 2], F32)
        nc.vector.tensor_copy(SPEC[:], SPEC_p[:])

        # ---- default-row derived constants ----
        # b1aug2 [1, 66] = [b1@W2 | b1.wt2s | b1.wt2d] (h2-space default row)
        b1aug2_p = psc.tile([1, D + 2], F32, space="PSUM")
        nc.tensor.matmul(b1aug2_p[:], b1cols[:], W2aug[:], start=True, stop=True)
        b1aug2 = const.tile([1, D + 2], F32)
        nc.vector.tensor_copy(b1aug2[:], b1aug2_p[:])
        # r1row [1, 64] = b1 @ W2 + b2
        r1row = const.tile([1, D], F32)
        nc.vector.tensor_tensor(r1row[:], b1aug2[0:1, 0:D], b2rows[:], op=OP.add)
        # broadcast across partitions via ones matmul
        ones1 = const.tile([1, P], F32)
        nc.vector.memset(ones1[:], 1.0)
        xdef_p = psc.tile([P, D + 2], F32, space="PSUM")
        nc.tensor.matmul(xdef_p[:], ones1[:], b1aug2[:], start=True, stop=True)
        xdefbc = const.tile([P, D + 2], F32)
        nc.vector.tensor_copy(xdefbc[:], xdef_p[:])
        b2_pb = psc.tile([P, D], F32, space="PSUM")
        nc.tensor.matmul(b2_pb[:], ones1[:], b2rows[:], start=True, stop=True)
        b2bc = const.tile([P, D], F32)
        nc.vector.tensor_copy(b2bc[:], b2_pb[:])
        r1_pb = psc.tile([P, D], F32, space="PSUM")
        nc.tensor.matmul(r1_pb[:], ones1[:], r1row[:], start=True, stop=True)
        # two full output images [P, J*64]: cols [0, NSB*64) are rewritten by
        # each iteration's special compute; cols [NSB*64, J*64) hold r1
        # (filled once by doubling).  Double-buffered so compute of iter i+1
        # overlaps the out DMA of iter i.
        J = meta["J"]
        bufA = const.tile([P, J * D], F32)
        bufB = const.tile([P, J * D], F32)
        for bt in (bufA, bufB):
            nc.vector.tensor_copy(bt[:, NSB * D:(NSB + 1) * D], r1_pb[:])
            w = 1
            while w < J - NSB:
                n = min(w, J - NSB - w)
                nc.vector.tensor_copy(bt[:, (NSB + w) * D:(NSB + w + n) * D],
                                      bt[:, NSB * D:(NSB + n) * D])
                w += n
        # xdefB [P, NSB*64] (h2_def row repeated per block)
        xdefB = const.tile([P, NSB * D], F32)
        nc.vector.tensor_copy(xdefB[:, 0:D], xdefbc[:, 0:D])
        w = 1
        while w < NSB:
            n = min(w, NSB - w)
            nc.vector.tensor_copy(xdefB[:, w * D:(w + n) * D], xdefB[:, 0:n * D])
            w += n
        # adstdefB [P, NSB] (default adst per block) by doubling
        adstdefB = const.tile([P, NSB], F32)
        nc.vector.tensor_copy(adstdefB[:, 0:1], xdefbc[:, D + 1:D + 2])
        w = 1
        while w < NSB:
            n = min(w, NSB - w)
            nc.vector.tensor_copy(adstdefB[:, w:w + n], adstdefB[:, 0:n])
            w += n

        psc_ctx.__exit__(None, None, None)

        # ---- layer 1: build h1 table for the U endpoint nodes ----
        with tc.tile_pool(name="l1u", bufs=2) as l1u, \
             tc.tile_pool(name="l1up", bufs=2, space="PSUM") as l1up:
            for t in range(nUt):
                xU = l1u.tile([P, D], F32, tag="xU")
                nc.sync.dma_start(xU[:], xu_in[t * P:(t + 1) * P, :])
                xT_p = l1up.tile([D, P], F32, space="PSUM", tag="xT")
                nc.tensor.transpose(xT_p[:], xU[:], ident[:])
                xT_s = l1u.tile([D, P], F32, tag="xTs")
                nc.vector.tensor_copy(xT_s[:], xT_p[:])
                h_p = l1up.tile([P, D + 2], F32, space="PSUM", tag="h_p")
                nc.tensor.matmul(h_p[:], xT_s[:], W1aug[:], start=True, stop=True)
                h_s = l1u.tile([P, P], F32, tag="h_s")
                nc.scalar.copy(h_s[:, 0:D + 2], h_p[:])
                nc.vector.memset(h_s[:, D + 2:P], 0.0)
                nc.sync.dma_start(h1tab[t * P:(t + 1) * P, :], h_s[:])

        # ---- layer 1 conv -> write table rows ----
        l1_eidx_s = const.tile([P, 8 * S1], I16)
        nc.sync.dma_start(l1_eidx_s[:], l1_eidx_in[:])
        l1_didx_s = const.tile([P, 8 * nblk1], I16)
        nc.sync.dma_start(l1_didx_s[:], l1_didx_in[:])
        l1_mask_s = const.tile([P, S1], F32)
        nc.sync.dma_start(l1_mask_s[:], l1_mask_in[:])
        l1_degpos_s = const.tile([P, nblk1], F32)
        nc.sync.dma_start(l1_degpos_s[:], l1_degpos_in[:])

        with tc.tile_pool(name="l1w", bufs=2) as l1w, \
             tc.tile_pool(name="l1p", bufs=2, space="PSUM") as l1p:
            dr1 = l1w.tile([P, nblk1, P], F32, tag="dr1")
            nc.gpsimd.dma_gather(dr1[:], h1tab[:, :], l1_didx_s[:],
                                 nblk1 * P, nblk1 * P, P, single_packet=False)
            adst1 = l1w.tile([P, nblk1], F32, tag="adst1")
            nc.scalar.activation(adst1[:],
                                 dr1[:, 0:nblk1, 65:66].rearrange("p b o -> p (b o)"),
                                 AF.Identity)
            for sb_i, sb in enumerate(sbs1):
                G1 = l1w.tile([P, sb["S"], P], F32, tag="G1")
                nc.gpsimd.dma_gather(
                    G1[:], h1tab[:, :],
                    l1_eidx_s[:, 8 * sb["slot0"]:8 * (sb["slot0"] + sb["S"])],
                    sb["S"] * P, sb["S"] * P, P, single_packet=False)
                for g in [g for g in groups1 if g["sb"] == sb_i]:
                    B, L, off = g["B"], g["L"], g["slot_off"]
                    sl0 = sb["slot0"] + off
                    msg = _emit_group(
                        nc, l1w, G1[:, off:off + B * L, :],
                        l1_mask_s[:, sl0:sl0 + B * L],
                        adst1[:, g["b0"]:g["b0"] + B],
                        l1_degpos_s[:, g["b0"]:g["b0"] + B], B, L)
                    for j in range(B):
                        b = g["b0"] + j
                        mT_p = l1p.tile([D, P], F32, space="PSUM", tag="mT")
                        nc.tensor.transpose(mT_p[:], msg[:, j, :], ident[:])
                        mT_s = l1w.tile([D + 1, P], F32, tag="mTs")
                        nc.vector.tensor_copy(mT_s[0:D, :], mT_p[:])
                        nc.vector.memset(mT_s[D:D + 1, :], 1.0)
                        row_p = l1p.tile([P, D + 2], F32, space="PSUM", tag="rowp")
                        nc.tensor.matmul(row_p[:], mT_s[:], SPEC[:],
                                         start=True, stop=True)
                        row_s = l1w.tile([P, P], F32, tag="rows")
                        nc.scalar.copy(row_s[:, 0:D + 2], row_p[:])
                        nc.vector.memset(row_s[:, D + 2:P], 0.0)
                        nrows = min(P, K - b * P)
                        if nrows > 0:
                            nc.sync.dma_start(
                                tab[1 + b * P:1 + b * P + nrows, :],
                                row_s[0:nrows, :])
                        if b == K // P:   # default row from the pad position K
                            q = K % P
                            nc.sync.dma_start(tab[0:1, :], row_s[q:q + 1, :])
                            # replicate the default row over rows K+1..VTAB-1
                            # (spreads default-row gather traffic across HBM)
                            zidx = l1w.tile([P, 8], I16, tag="zidx")
                            nc.vector.memset(zidx[:], 0)
                            defbc = l1w.tile([P, 1, P], F32, tag="defbc")
                            nc.gpsimd.dma_gather(defbc[:], tab[:, :], zidx[:],
                                                 P, P, P, single_packet=False)
                            r0 = K + 1
                            while r0 < VTAB:
                                cnt = min(P, VTAB - r0)
                                nc.sync.dma_start(tab[r0:r0 + cnt, :],
                                                  defbc[0:cnt, 0, :])
                                r0 += cnt

        # ---- layer 2 ----
        eidx_s = const.tile([P, 8 * (S2 + NCOL)], I16)
        nc.sync.dma_start(eidx_s[:], eidx_in[:])
        iidx_s = const.tile([P, S2 + NCOL], I32)
        nc.sync.dma_start(iidx_s[:], iidx_in[:])
        mask_s = const.tile([P, S2], F32)
        nc.sync.dma_start(mask_s[:], mask_in[:])
        cdef_s = const.tile([P, NSB], F32)
        nc.sync.dma_start(cdef_s[:], cdef_in[:])
        fsel_s = const.tile([P, NCOL * NSB], F32)
        nc.sync.dma_start(fsel_s[:], fsel_in[:])
        kinv_s = const.tile([P, NCOL * NSB], F32)
        nc.sync.dma_start(kinv_s[:], kinv_in[:])

        with tc.tile_pool(name="gw", bufs=bufs) as gw, \
             tc.tile_pool(name="blk", bufs=bufs) as blk:
            SG = S2 + NCOL
            J = meta["J"]
            out_v = out_t[0:npos, :].rearrange("(p j) f -> p (j f)", p=P)
            for _rep in range(repeat):
                buf = bufA if _rep % 2 == 0 else bufB
                if "gather" not in parts:
                    if "bcast" in parts:
                        nc.sync.dma_start(out_v, buf[:])
                    continue
                Gall = gw.tile([P, SG, P], F32, tag="Gall")
                if use_indirect:
                    nc.gpsimd.indirect_dma_start(
                        out=Gall[:], out_offset=None, in_=tab[:, :],
                        in_offset=bass.IndirectOffsetOnAxis(ap=iidx_s[:], axis=0))
                else:
                    nc.gpsimd.dma_gather(Gall[:], tab[:, :], eidx_s[:, 0:8 * SG],
                                         SG * P, SG * P, P,
                                         single_packet=single_packet)
                if "compute" not in parts:
                    dum = gw.tile([P, P], F32, tag="dum")
                    nc.vector.tensor_copy(dum[:], Gall[:, 0, :])
                    if "bcast" in parts:
                        nc.sync.dma_start(out_v, buf[:])
                    continue
                # adst: analytic default + gathered fixups
                prev = adstdefB[:]
                for q in range(NCOL):
                    aq = gw.tile([P, NSB], F32, tag=f"a{q}")
                    nc.vector.tensor_tensor(aq[:], prev,
                                            kinv_s[:, q * NSB:(q + 1) * NSB],
                                            op=OP.mult)
                    fq = gw.tile([P, NSB], F32, tag=f"f{q}")
                    nc.vector.tensor_tensor(
                        fq[:], fsel_s[:, q * NSB:(q + 1) * NSB],
                        Gall[:, S2 + q, 65:66].to_broadcast((P, NSB)),
                        op=OP.mult)
                    mq = gw.tile([P, NSB], F32, tag=f"m{q}")
                    nc.vector.tensor_tensor(mq[:], aq[:], fq[:], op=OP.add)
                    prev = mq[:]
                adst = prev
                # default-mass weight: wd = cdef * exp(leaky(asrc_def + adst))
                zt = gw.tile([P, NSB], F32, tag="zt")
                nc.vector.tensor_tensor(
                    zt[:], adst[:],
                    xdefbc[:, D:D + 1].to_broadcast((P, NSB)), op=OP.add)
                ut = gw.tile([P, NSB], F32, tag="ut")
                nc.vector.scalar_tensor_tensor(ut[:], zt[:], NEG_SLOPE, zt[:],
                                               op0=OP.mult, op1=OP.max)
                exd = gw.tile([P, NSB], F32, tag="exd")
                nc.scalar.activation(exd[:], ut[:], AF.Exp)
                wd = gw.tile([P, NSB], F32, tag="wd")
                nc.vector.tensor_tensor(wd[:], cdef_s[:], exd[:], op=OP.mult)

                bufv = buf[:].rearrange("p (j f) -> p j f", f=D)
                for g in groups2:
                    B, L, off = g["B"], g["L"], g["slot_off"]
                    b0 = g["b0"]
                    _emit_special(
                        nc, gw, Gall[:, off:off + B * L, :],
                        mask_s[:, off:off + B * L],
                        adst[:, b0:b0 + B], wd[:, b0:b0 + B],
                        cdef_s[:, b0:b0 + B],
                        xdefB[:].rearrange("p (b f) -> p b f", f=D)
                        [:, b0:b0 + B, :],
                        b2bc[:], bufv[:, b0:b0 + B, :], B, L)
                # one write: row p*J+j <- buf[p, j] (specials j<NSB, r1 rest)
                if "bcast" in parts:
                    nc.sync.dma_start(out_v, buf[:])

    nc.compile()
    return nc


def make_in_maps(inputs, meta, l1, cores):
    x = np.asarray(inputs["x"], dtype=np.float32)
    W1 = np.asarray(inputs["W1"], dtype=np.float32)
    W2 = np.asarray(inputs["W2"], dtype=np.float32)
    base = {
        "xu_in": np.ascontiguousarray(x[l1["upad"]]),
        "W1_in": np.ascontiguousarray(W1),
        "W1T_in": np.ascontiguousarray(W1.T),
        "W2_in": np.ascontiguousarray(W2),
        "W2T_in": np.ascontiguousarray(W2.T),
        "av1_in": np.ascontiguousarray(np.stack(
            [np.asarray(inputs["a_src1"]), np.asarray(inputs["a_dst1"])],
            axis=1).astype(np.float32)),
        "av2_in": np.ascontiguousarray(np.stack(
            [np.asarray(inputs["a_src2"]), np.asarray(inputs["a_dst2"])],
            axis=1).astype(np.float32)),
        "b1row_in": np.asarray(inputs["b1"], dtype=np.float32).reshape(1, D),
        "b2row_in": np.asarray(inputs["b2"], dtype=np.float32).reshape(1, D),
        "b1col_in": np.asarray(inputs["b1"], dtype=np.float32).reshape(D, 1),
        "l1_eidx_in": l1["l1_eidx"],
        "l1_didx_in": l1["l1_didx"],
        "l1_mask_in": l1["l1_mask"],
        "l1_degpos_in": l1["l1_degpos"],
    }
    in_maps = []
    for c in range(NCORES):
        m = dict(base)
        m["eidx_in"] = cores[c]["eidx"]
        m["iidx_in"] = cores[c]["iidx"]
        m["mask_in"] = cores[c]["mask"]
        m["cdef_in"] = cores[c]["cdef"]
        m["fsel_in"] = cores[c]["fsel"]
        m["kinv_in"] = cores[c]["kinv"]
        in_maps.append(m)
    return in_maps


def unshard_core(oc, core):
    """Per-core [npos, D] device rows -> [NPC, D] node-ordered block."""
    blk = np.empty((NPC, D), np.float32)
    blk[core["nodes"]] = oc[core["rows"]]
    return blk


def unshard(results, cores):
    out = np.empty((N, D), np.float32)
    for c in range(NCORES):
        out[c * NPC:(c + 1) * NPC] = unshard_core(results[c]["out"], cores[c])
    return out


def kernel(**inputs):
    meta, l1, cores = prep(inputs)
    nc = build(meta, repeat=1)
    in_maps = make_in_maps(inputs, meta, l1, cores)
    res = run_bass_kernel_spmd(nc, in_maps, core_ids=list(range(NCORES)))
    return unshard(res.results, cores)
